# revision 14
# baseline (speedup 1.0000x reference)
"""Trainium2 Bass kernel for CoordinateMassivePool.

Per batch element b:
    center = mu[b] * (TOTAL-1)
    start  = clip(center - W//2, 0, TOTAL-W) (int32)            -> output 2
    window = params[start:start+W]                (W=256 rows)
    w_j    = exp(-(start+j-center)^2 / (2*(sigma[b]+1e-6)^2))
    out[b] = sum_j w_j * window[j] / (sum_j w_j + 1e-6)         -> output 1

Key optimization: sigma < 1, so the Gaussian weight support is < +-7.5
rows around `center`; everything else underflows f32 accumulation. Each
batch only needs the 12-row sub-window [t, t+12) with
    t = clip(round(center - 0.5) - 5, 0, TOTAL - 12).
Containment in the reference window ([t, t+12) subset of
[start, start+256) and of [0, TOTAL)) holds for all centers, and the
largest excluded weight is exp(-6^2/2) ~ 1.5e-8 relative. This cuts the
gather from 512 KB/batch of window reads to 6 KB/batch (537 MB ->
25 MB total), and `t` needs no floor and no dependence on `start`,
so the gather-index critical path is 5 small vector ops.

Sharding: data-parallel over B. Each of the 8 cores handles 512 batch
elements and a full replica of the (1M, 128) table in its HBM. The
12-row windows are fetched with one indirect (index-gathering) DMA per
128-batch group: each int32 row index pulls a contiguous 6 KB block
into one SBUF partition.

Batch layout on a core: b = p*4 + q (p = SBUF partition, q = group).
"""

import numpy as np

import concourse.bass as bass
import concourse.mybir as mybir
import concourse.tile as tile
from concourse import bacc, bass_utils
from concourse.bass import IndirectOffsetOnAxis

TOTAL = 1_000_000
D = 128
B = 4096
W = 256
N_CORES = 8
BPC = B // N_CORES  # 512 batches per core
P = 128             # SBUF partitions
Q = BPC // P        # 4 groups per core
WIN = 10            # truncated window rows per batch
OFF = 4             # t = round(center-0.5) - OFF

F32 = mybir.dt.float32
I32 = mybir.dt.int32
MAGIC = 8388608.0   # 2^23: f32 round-to-nearest-int magic constant


def build_kernel():
    nc = bacc.Bacc("TRN2", target_bir_lowering=False, debug=False)

    mu = nc.dram_tensor("mu", [BPC], F32, kind="ExternalInput")
    sigma = nc.dram_tensor("sigma", [BPC], F32, kind="ExternalInput")
    params = nc.dram_tensor("params_storage", [TOTAL, D], F32,
                            kind="ExternalInput")
    out_agg = nc.dram_tensor("out_agg", [BPC, D], F32, kind="ExternalOutput")
    out_start = nc.dram_tensor("out_start", [BPC], I32, kind="ExternalOutput")

    AL = mybir.AluOpType

    with tile.TileContext(nc) as tc:
        with (
            tc.tile_pool(name="small", bufs=1) as sp,
            tc.tile_pool(name="gather", bufs=4) as gp,
            tc.tile_pool(name="work", bufs=2) as wp,
        ):
            # ---- load mu/sigma as (128, 4): b = p*4 + q ----
            mu_t = sp.tile([P, Q], F32)
            sg_t = sp.tile([P, Q], F32)
            nc.sync.dma_start(mu_t[:], mu[:].rearrange("(p q) -> p q", q=Q))
            nc.sync.dma_start(sg_t[:], sigma[:].rearrange("(p q) -> p q", q=Q))

            # ---- gather index t (critical path, 3 fused ops) ----
            # t = clip(round_half_even(center - 0.5) - OFF, 0, TOTAL - WIN)
            tm = sp.tile([P, Q], F32)
            nc.vector.tensor_scalar(tm[:], mu_t[:], float(TOTAL - 1),
                                    MAGIC - 0.5, op0=AL.mult, op1=AL.add)
            t3 = sp.tile([P, Q], F32)
            nc.vector.tensor_scalar(t3[:], tm[:], MAGIC + OFF, 0.0,
                                    op0=AL.subtract, op1=AL.max)
            t_i = sp.tile([P, Q], I32)
            nc.vector.tensor_scalar(t_i[:], t3[:], float(TOTAL - WIN), None,
                                    op0=AL.min)
            # off the critical path: f32 copies needed by the weight math
            c = sp.tile([P, Q], F32)
            nc.vector.tensor_scalar_mul(c[:], mu_t[:], float(TOTAL - 1))
            t_f = sp.tile([P, Q], F32)
            nc.vector.tensor_scalar(t_f[:], t3[:], float(TOTAL - WIN), None,
                                    op0=AL.min)

            # ---- issue all 4 gathers as soon as t is ready ----
            g_tiles = []
            with tc.high_priority():
                for q in range(Q):
                    g = gp.tile([P, WIN * D], F32, tag="g")
                    nc.gpsimd.indirect_dma_start(
                        out=g[:],
                        out_offset=None,
                        in_=params[:],
                        in_offset=IndirectOffsetOnAxis(ap=t_i[:, q:q + 1],
                                                       axis=0),
                    )
                    g_tiles.append(g)

            # ---- start_indices = floor(clip(center-128, 0, TOTAL-W)) ----
            # (exact; off the gather critical path)
            sf = sp.tile([P, Q], F32)
            nc.vector.tensor_scalar(sf[:], c[:], -float(W // 2), 0.0,
                                    op0=AL.add, op1=AL.max)
            nc.vector.tensor_scalar(sf[:], sf[:], float(TOTAL - W), None,
                                    op0=AL.min)
            m1 = sp.tile([P, Q], F32)
            nc.vector.tensor_scalar_add(m1[:], sf[:], MAGIC)
            m = sp.tile([P, Q], F32)
            nc.vector.tensor_scalar_add(m[:], m1[:], -MAGIC)
            corr = sp.tile([P, Q], F32)
            nc.vector.tensor_tensor(out=corr[:], in0=m[:], in1=sf[:],
                                    op=AL.is_gt)
            nc.vector.tensor_sub(m[:], m[:], corr[:])
            s_i = sp.tile([P, Q], I32)
            nc.vector.tensor_copy(s_i[:], m[:])
            nc.sync.dma_start(out_start[:].rearrange("(p q) -> p q", q=Q),
                              s_i[:])

            # ---- weights w[p, q*WIN+j] = exp(-(t+j-c)^2/(2*(sig+1e-6)^2))
            jf = sp.tile([P, Q * WIN], F32)
            nc.gpsimd.iota(jf[:].rearrange("p (a b) -> p a b", b=WIN),
                           pattern=[[0, Q], [1, WIN]], channel_multiplier=0,
                           allow_small_or_imprecise_dtypes=True)
            tmc = sp.tile([P, Q], F32)
            nc.vector.tensor_sub(tmc[:], t_f[:], c[:])
            d = sp.tile([P, Q * WIN], F32)
            nc.vector.tensor_tensor(
                out=d[:].rearrange("p (a b) -> p a b", b=WIN),
                in0=tmc[:].unsqueeze(2).to_broadcast([P, Q, WIN]),
                in1=jf[:].rearrange("p (a b) -> p a b", b=WIN),
                op=AL.add)
            sigp = sp.tile([P, Q], F32)
            nc.vector.tensor_scalar_add(sigp[:], sg_t[:], 1e-6)
            den = sp.tile([P, Q], F32)
            nc.vector.tensor_mul(den[:], sigp[:], sigp[:])
            nc.vector.tensor_scalar_mul(den[:], den[:], 2.0)
            nrcp = sp.tile([P, Q], F32)
            nc.vector.reciprocal(nrcp[:], den[:])
            nc.vector.tensor_scalar_mul(nrcp[:], nrcp[:], -1.0)
            arg = sp.tile([P, Q * WIN], F32)
            nc.vector.tensor_mul(arg[:], d[:], d[:])
            nc.vector.tensor_tensor(
                out=arg[:].rearrange("p (a b) -> p a b", b=WIN),
                in0=arg[:].rearrange("p (a b) -> p a b", b=WIN),
                in1=nrcp[:].unsqueeze(2).to_broadcast([P, Q, WIN]),
                op=AL.mult)
            w = sp.tile([P, Q * WIN], F32)
            nc.scalar.activation(w[:], arg[:], mybir.ActivationFunctionType.Exp)

            # ---- normalizer 1 / (sum_j w + 1e-6) ----
            ssum = sp.tile([P, Q], F32)
            nc.vector.reduce_sum(ssum[:],
                                 w[:].rearrange("p (a b) -> p a b", b=WIN),
                                 axis=mybir.AxisListType.X)
            nc.vector.tensor_scalar_add(ssum[:], ssum[:], 1e-6)
            rn = sp.tile([P, Q], F32)
            nc.vector.reciprocal(rn[:], ssum[:])
            wn = sp.tile([P, Q * WIN], F32)
            nc.vector.tensor_tensor(
                out=wn[:].rearrange("p (a b) -> p a b", b=WIN),
                in0=w[:].rearrange("p (a b) -> p a b", b=WIN),
                in1=rn[:].unsqueeze(2).to_broadcast([P, Q, WIN]),
                op=AL.mult)

            # ---- per group: weighted reduce over the 12 window rows ----
            agg3 = out_agg[:].rearrange("(p q) d -> p q d", q=Q)
            for q in range(Q):
                g = g_tiles[q]
                tmp = wp.tile([P, WIN * D], F32, tag="tmp")
                nc.vector.tensor_tensor(
                    out=tmp[:].rearrange("p (j d) -> p j d", d=D),
                    in0=g[:].rearrange("p (j d) -> p j d", d=D),
                    in1=wn[:, q * WIN:(q + 1) * WIN].unsqueeze(2)
                        .to_broadcast([P, WIN, D]),
                    op=AL.mult)
                a1 = wp.tile([P, 5 * D], F32, tag="a1")
                # group 2's first-level add runs on GpSimd, which is free
                # once the SWDGE drain completes (~22us) — frees ~0.8us of
                # Vector-engine time in the saturated main phase.
                a1_eng = nc.gpsimd if q == 2 else nc.vector
                a1_eng.tensor_add(a1[:], tmp[:, :5 * D], tmp[:, 5 * D:])
                a2 = wp.tile([P, 2 * D], F32, tag="a2")
                nc.vector.tensor_add(a2[:], a1[:, :2 * D], a1[:, 2 * D:4 * D])
                u = wp.tile([P, D], F32, tag="u")
                nc.vector.tensor_add(u[:], a2[:, :D], a2[:, D:])
                nc.vector.tensor_add(u[:], u[:], a1[:, 4 * D:])
                nc.sync.dma_start(agg3[:, q, :], u[:])

    nc.compile()
    return nc


_NC_CACHE = []


def _get_nc():
    if not _NC_CACHE:
        _NC_CACHE.append(build_kernel())
    return _NC_CACHE[0]


def run_spmd(mu, sigma, params_storage, trace=False):
    nc = _get_nc()
    mu = np.ascontiguousarray(np.asarray(mu, dtype=np.float32))
    sigma = np.ascontiguousarray(np.asarray(sigma, dtype=np.float32))
    params_storage = np.ascontiguousarray(
        np.asarray(params_storage, dtype=np.float32))
    in_maps = [
        {
            "mu": mu[c * BPC:(c + 1) * BPC],
            "sigma": sigma[c * BPC:(c + 1) * BPC],
            "params_storage": params_storage,
        }
        for c in range(N_CORES)
    ]
    res = bass_utils.run_bass_kernel_spmd(
        nc, in_maps, core_ids=list(range(N_CORES)), trace=trace)
    agg = np.concatenate([r["out_agg"] for r in res.results], axis=0)
    starts = np.concatenate([r["out_start"] for r in res.results], axis=0)
    return (agg, starts.astype(np.int32)), res


def kernel(mu, sigma, params_storage):
    (agg, starts), _ = run_spmd(mu, sigma, params_storage, trace=False)
    return agg, starts


# revision 15
# speedup vs baseline: 1.0569x; 1.0569x over previous
"""Trainium2 Bass kernel for CoordinateMassivePool.

Per batch element b:
    center = mu[b] * (TOTAL-1)
    start  = clip(center - W//2, 0, TOTAL-W) (int32)            -> output 2
    window = params[start:start+W]                (W=256 rows)
    w_j    = exp(-(start+j-center)^2 / (2*(sigma[b]+1e-6)^2))
    out[b] = sum_j w_j * window[j] / (sum_j w_j + 1e-6)         -> output 1

Key optimization: sigma < 1, so the Gaussian weight support is < +-7.5
rows around `center`; everything else underflows f32 accumulation. Each
batch only needs the 12-row sub-window [t, t+12) with
    t = clip(round(center - 0.5) - 5, 0, TOTAL - 12).
Containment in the reference window ([t, t+12) subset of
[start, start+256) and of [0, TOTAL)) holds for all centers, and the
largest excluded weight is exp(-6^2/2) ~ 1.5e-8 relative. This cuts the
gather from 512 KB/batch of window reads to 6 KB/batch (537 MB ->
25 MB total), and `t` needs no floor and no dependence on `start`,
so the gather-index critical path is 5 small vector ops.

Sharding: data-parallel over B. Each of the 8 cores handles 512 batch
elements and a full replica of the (1M, 128) table in its HBM. The
12-row windows are fetched with one indirect (index-gathering) DMA per
128-batch group: each int32 row index pulls a contiguous 6 KB block
into one SBUF partition.

Batch layout on a core: b = p*4 + q (p = SBUF partition, q = group).
"""

import numpy as np

import concourse.bass as bass
import concourse.mybir as mybir
import concourse.tile as tile
from concourse import bacc, bass_utils
from concourse.bass import IndirectOffsetOnAxis

TOTAL = 1_000_000
D = 128
B = 4096
W = 256
N_CORES = 8
BPC = B // N_CORES  # 512 batches per core
P = 128             # SBUF partitions
Q = BPC // P        # 4 groups per core
WIN = 10            # truncated window rows per batch
OFF = 4             # t = round(center-0.5) - OFF

F32 = mybir.dt.float32
I32 = mybir.dt.int32
MAGIC = 8388608.0   # 2^23: f32 round-to-nearest-int magic constant


def build_kernel():
    nc = bacc.Bacc("TRN2", target_bir_lowering=False, debug=False)

    mu = nc.dram_tensor("mu", [BPC], F32, kind="ExternalInput")
    sigma = nc.dram_tensor("sigma", [BPC], F32, kind="ExternalInput")
    params = nc.dram_tensor("params_storage", [TOTAL, D], F32,
                            kind="ExternalInput")
    out_agg = nc.dram_tensor("out_agg", [BPC, D], F32, kind="ExternalOutput")
    out_start = nc.dram_tensor("out_start", [BPC], I32, kind="ExternalOutput")

    AL = mybir.AluOpType

    with tile.TileContext(nc) as tc:
        with (
            tc.tile_pool(name="small", bufs=1) as sp,
            tc.tile_pool(name="gather", bufs=4) as gp,
            tc.tile_pool(name="work", bufs=2) as wp,
        ):
            # ---- load mu/sigma as (128, 4): b = p*4 + q ----
            mu_t = sp.tile([P, Q], F32)
            sg_t = sp.tile([P, Q], F32)
            nc.sync.dma_start(mu_t[:], mu[:].rearrange("(p q) -> p q", q=Q))
            nc.sync.dma_start(sg_t[:], sigma[:].rearrange("(p q) -> p q", q=Q))

            # ---- gather index t (critical path, 3 fused ops) ----
            # t = clip(round_half_even(center - 0.5) - OFF, 0, TOTAL - WIN)
            tm = sp.tile([P, Q], F32)
            nc.vector.tensor_scalar(tm[:], mu_t[:], float(TOTAL - 1),
                                    MAGIC - 0.5, op0=AL.mult, op1=AL.add)
            t3 = sp.tile([P, Q], F32)
            nc.vector.tensor_scalar(t3[:], tm[:], MAGIC + OFF, 0.0,
                                    op0=AL.subtract, op1=AL.max)
            t_i = sp.tile([P, Q], I32)
            nc.vector.tensor_scalar(t_i[:], t3[:], float(TOTAL - WIN), None,
                                    op0=AL.min)
            # off the critical path: f32 copies needed by the weight math
            c = sp.tile([P, Q], F32)
            nc.vector.tensor_scalar_mul(c[:], mu_t[:], float(TOTAL - 1))
            t_f = sp.tile([P, Q], F32)
            nc.vector.tensor_scalar(t_f[:], t3[:], float(TOTAL - WIN), None,
                                    op0=AL.min)

            # ---- issue all 4 gathers as soon as t is ready ----
            g_tiles = []
            with tc.high_priority():
                for q in range(Q):
                    g = gp.tile([P, WIN * D], F32, tag="g")
                    nc.gpsimd.indirect_dma_start(
                        out=g[:],
                        out_offset=None,
                        in_=params[:],
                        in_offset=IndirectOffsetOnAxis(ap=t_i[:, q:q + 1],
                                                       axis=0),
                    )
                    g_tiles.append(g)

            # ---- start_indices = floor(clip(center-128, 0, TOTAL-W)) ----
            # (exact; off the gather critical path)
            sf = sp.tile([P, Q], F32)
            nc.vector.tensor_scalar(sf[:], c[:], -float(W // 2), 0.0,
                                    op0=AL.add, op1=AL.max)
            nc.vector.tensor_scalar(sf[:], sf[:], float(TOTAL - W), None,
                                    op0=AL.min)
            m1 = sp.tile([P, Q], F32)
            nc.vector.tensor_scalar_add(m1[:], sf[:], MAGIC)
            m = sp.tile([P, Q], F32)
            nc.vector.tensor_scalar_add(m[:], m1[:], -MAGIC)
            corr = sp.tile([P, Q], F32)
            nc.vector.tensor_tensor(out=corr[:], in0=m[:], in1=sf[:],
                                    op=AL.is_gt)
            nc.vector.tensor_sub(m[:], m[:], corr[:])
            s_i = sp.tile([P, Q], I32)
            nc.vector.tensor_copy(s_i[:], m[:])
            nc.sync.dma_start(out_start[:].rearrange("(p q) -> p q", q=Q),
                              s_i[:])

            # ---- weights w[p, q*WIN+j] = exp(-(t+j-c)^2/(2*(sig+1e-6)^2))
            jf = sp.tile([P, Q * WIN], F32)
            nc.gpsimd.iota(jf[:].rearrange("p (a b) -> p a b", b=WIN),
                           pattern=[[0, Q], [1, WIN]], channel_multiplier=0,
                           allow_small_or_imprecise_dtypes=True)
            tmc = sp.tile([P, Q], F32)
            nc.vector.tensor_sub(tmc[:], t_f[:], c[:])
            d = sp.tile([P, Q * WIN], F32)
            nc.vector.tensor_tensor(
                out=d[:].rearrange("p (a b) -> p a b", b=WIN),
                in0=tmc[:].unsqueeze(2).to_broadcast([P, Q, WIN]),
                in1=jf[:].rearrange("p (a b) -> p a b", b=WIN),
                op=AL.add)
            sigp = sp.tile([P, Q], F32)
            nc.vector.tensor_scalar_add(sigp[:], sg_t[:], 1e-6)
            den = sp.tile([P, Q], F32)
            nc.vector.tensor_mul(den[:], sigp[:], sigp[:])
            nc.vector.tensor_scalar_mul(den[:], den[:], 2.0)
            nrcp = sp.tile([P, Q], F32)
            nc.vector.reciprocal(nrcp[:], den[:])
            nc.vector.tensor_scalar_mul(nrcp[:], nrcp[:], -1.0)
            arg = sp.tile([P, Q * WIN], F32)
            nc.vector.tensor_mul(arg[:], d[:], d[:])
            nc.vector.tensor_tensor(
                out=arg[:].rearrange("p (a b) -> p a b", b=WIN),
                in0=arg[:].rearrange("p (a b) -> p a b", b=WIN),
                in1=nrcp[:].unsqueeze(2).to_broadcast([P, Q, WIN]),
                op=AL.mult)
            w = sp.tile([P, Q * WIN], F32)
            nc.scalar.activation(w[:], arg[:], mybir.ActivationFunctionType.Exp)

            # ---- normalizer 1 / (sum_j w + 1e-6) ----
            ssum = sp.tile([P, Q], F32)
            nc.vector.reduce_sum(ssum[:],
                                 w[:].rearrange("p (a b) -> p a b", b=WIN),
                                 axis=mybir.AxisListType.X)
            nc.vector.tensor_scalar_add(ssum[:], ssum[:], 1e-6)
            rn = sp.tile([P, Q], F32)
            nc.vector.reciprocal(rn[:], ssum[:])
            wn = sp.tile([P, Q * WIN], F32)
            nc.vector.tensor_tensor(
                out=wn[:].rearrange("p (a b) -> p a b", b=WIN),
                in0=w[:].rearrange("p (a b) -> p a b", b=WIN),
                in1=rn[:].unsqueeze(2).to_broadcast([P, Q, WIN]),
                op=AL.mult)

            # ---- per group: weighted reduce over the 12 window rows ----
            agg3 = out_agg[:].rearrange("(p q) d -> p q d", q=Q)
            for q in range(Q):
                g = g_tiles[q]
                tmp = wp.tile([P, WIN * D], F32, tag="tmp")
                nc.vector.tensor_tensor(
                    out=tmp[:].rearrange("p (j d) -> p j d", d=D),
                    in0=g[:].rearrange("p (j d) -> p j d", d=D),
                    in1=wn[:, q * WIN:(q + 1) * WIN].unsqueeze(2)
                        .to_broadcast([P, WIN, D]),
                    op=AL.mult)
                a1 = wp.tile([P, 5 * D], F32, tag="a1")
                nc.vector.tensor_add(a1[:], tmp[:, :5 * D], tmp[:, 5 * D:])
                a2 = wp.tile([P, 2 * D], F32, tag="a2")
                nc.vector.tensor_add(a2[:], a1[:, :2 * D], a1[:, 2 * D:4 * D])
                u = wp.tile([P, D], F32, tag="u")
                nc.vector.tensor_add(u[:], a2[:, :D], a2[:, D:])
                nc.vector.tensor_add(u[:], u[:], a1[:, 4 * D:])
                nc.sync.dma_start(agg3[:, q, :], u[:])

    nc.compile()
    return nc


_NC_CACHE = []


def _get_nc():
    if not _NC_CACHE:
        _NC_CACHE.append(build_kernel())
    return _NC_CACHE[0]


def run_spmd(mu, sigma, params_storage, trace=False):
    nc = _get_nc()
    mu = np.ascontiguousarray(np.asarray(mu, dtype=np.float32))
    sigma = np.ascontiguousarray(np.asarray(sigma, dtype=np.float32))
    params_storage = np.ascontiguousarray(
        np.asarray(params_storage, dtype=np.float32))
    in_maps = [
        {
            "mu": mu[c * BPC:(c + 1) * BPC],
            "sigma": sigma[c * BPC:(c + 1) * BPC],
            "params_storage": params_storage,
        }
        for c in range(N_CORES)
    ]
    res = bass_utils.run_bass_kernel_spmd(
        nc, in_maps, core_ids=list(range(N_CORES)), trace=trace)
    agg = np.concatenate([r["out_agg"] for r in res.results], axis=0)
    starts = np.concatenate([r["out_start"] for r in res.results], axis=0)
    return (agg, starts.astype(np.int32)), res


def kernel(mu, sigma, params_storage):
    (agg, starts), _ = run_spmd(mu, sigma, params_storage, trace=False)
    return agg, starts
